# revision 2
# baseline (speedup 1.0000x reference)
"""Self-contained Trainium2 Bass kernel for nn_CAELoss (loss_fn).

Contract: kernel(**inputs) takes the FULL unsharded inputs
(x [4096,3072], x_hat [4096,3072], target [4096] i32, z_in [4096,128],
z_out [4096,128], center_arr [10,128]) and returns the FULL output
(scalar f32 loss).

Strategy (data-parallel over batch, 8 NeuronCores), memory-bound:
  - x/x_hat stream in fp8e4m3 as ONE fused tensor; most columns are
    consumed by PE gram matmuls ([x64|xh64] blocks self-matmul'd into one
    PSUM accumulator; diag = sum x^2 + sum xh^2, +64 off-diag = sum x*xh,
    extracted with eye masks), the rest by a DVE-sub + ACT-square path.
    PE is ~4x cheaper per byte than DVE+ACT, so rt3 is all-PE and the
    stream ENDS with a tiny 3-block gram chunk: the post-stream tail is
    just 0.2us of PE + two eye-extract STTs + the stats DMA.
  - per-rt [gram | ve] interleave keeps the DVE/ACT pipeline fed evenly
    through the stream (mid-stream DMA completion sems lag the data by
    ~1.5-2us due to 16-engine skew; keeping DVE/ACT at ~50% load means
    that lag never accumulates).
  - z path batched: one [10,512] matmul of centers against all 512 z_in
    rows (+ a ones-matmul folding in -(|z|^2+1)/2), PE-transposed back
    to [128,10] tiles, one sqrt per tile, tiny DVE tail for pos/neg.
  - z_out rides the fp8 tensor (|z|^2 only needs ~1% accuracy);
    constants/z_in ride ONE fused bf16 tensor (single DMA).
  - all DMA issue rides the sync HWDGE ring in completion-order.
  - device emits a [128, NSTAT] tile of per-partition partial sums;
    host reduces the 8x128 partials to the scalar loss.
"""

import sys

import numpy as np

if "/opt/trn_rl_repo" not in sys.path:
    sys.path.insert(0, "/opt/trn_rl_repo")

import ml_dtypes

B, D, C, L = 4096, 3072, 10, 128
N_CORES = 8
BS = B // N_CORES  # 512 batch rows per core
P = 128  # SBUF partitions
NT = BS // P  # 4 row tiles of 128 rows per core

# gram blocks: 64 feature cols each, packed [x64|xh64] = 128 bytes
BLK012 = 37  # blocks per row-tile 0..2 (feature cols [0, 2368))
BLK3 = 48  # row-tile 3 is all-PE (feature cols [0, 3072))
BLK2X = 3  # rt2 extra blocks (feature cols [2880, 3072)), streamed LAST
PE_W = BLK012 * 64  # 2368
VE_W01 = D - PE_W  # 704 ve cols for rt0/rt1
VE_W2 = 512  # rt2 ve cols [2368, 2880); [2880,3072) goes via g2x

GW = BLK012 * 128  # 4736 gram bytes per rt0..2 line
VB01 = 2 * VE_W01  # 1408
VB2 = 2 * VE_W2  # 1024
G3W = BLK3 * 128  # 6144
G2XW = BLK2X * 128  # 384

# xx per-partition layout (f8 bytes)
O_G0 = 0
O_VE0 = O_G0 + GW  # 4736
O_G1 = O_VE0 + VB01  # 6144
O_VE1 = O_G1 + GW  # 10880
O_ZO = O_VE1 + VB01  # 12288
O_G2 = O_ZO + NT * L  # 12800
O_VE2 = O_G2 + GW  # 17536
O_G3 = O_VE2 + VB2  # 18560
O_G2X = O_G3 + G3W  # 24704
XW = O_G2X + G2XW  # 25088

# stats columns: 0 gram-eye | 1 gram-shift | [2:5] ve | tc NT | ol NT | orth
NVE = 3
C_VE = 2
C_TC = C_VE + NVE  # 5
C_OL = C_TC + NT  # 9
C_OR = C_OL + NT  # 13
NSTAT = C_OR + 1  # 14

# bcat (bf16) fused constant/z_in layout
O_Z = 0  # zin transposed [128, 512]
O_CEN = NT * P  # 512
O_ONE = O_CEN + C  # 522
O_ONE10 = O_ONE + 1  # 523
O_OH = O_ONE10 + C  # 533
O_EYEI = O_OH + NT * C  # 573
O_EYES = O_EYEI + P  # 701
O_EYE10 = O_EYES + P  # 829
O_OHB = O_EYE10 + C  # 839
BW = O_OHB + NT * C  # 879

D_IN = 0.1
BIG = 1.0e9

_CACHE = {}


def _build():
    """Build + compile the single-core SPMD Bass program."""
    from contextlib import ExitStack

    import concourse.bacc as bacc
    import concourse.mybir as mybir
    import concourse.tile as tile

    f32 = mybir.dt.float32
    bf16 = mybir.dt.bfloat16
    f8 = mybir.dt.float8e4
    Alu = mybir.AluOpType
    Act = mybir.ActivationFunctionType

    nc = bacc.Bacc(
        "TRN2",
        target_bir_lowering=False,
        debug=False,
        enable_asserts=True,
        num_devices=N_CORES,
    )

    xx_d = nc.dram_tensor("xx", [P, XW], f8, kind="ExternalInput")
    bcat_d = nc.dram_tensor("bcat", [P, BW], bf16, kind="ExternalInput")
    out_d = nc.dram_tensor("out", [P, NSTAT], f32, kind="ExternalOutput")

    with tile.TileContext(nc) as tc, ExitStack() as ctx:
        st = ctx.enter_context(tc.tile_pool(name="st", bufs=1))
        sp = ctx.enter_context(tc.tile_pool(name="sp", bufs=1))
        pp = ctx.enter_context(tc.tile_pool(name="pp", bufs=1, space="PSUM"))

        # ---- DMA issue: ALL on the sync HWDGE ring, issue order ==
        # address order == completion order (FIFO per ring).
        bcat = st.tile([P, BW], bf16)
        nc.sync.dma_start(bcat[:], bcat_d[:])

        def load(off, width, tag):
            t = st.tile([P, width], f8, tag=tag)
            nc.sync.dma_start(t[:], xx_d[:, off : off + width])
            return t

        g0 = load(O_G0, GW, "g0")
        ve0 = load(O_VE0, VB01, "ve0")
        g1 = load(O_G1, GW, "g1")
        ve1 = load(O_VE1, VB01, "ve1")
        zot = load(O_ZO, NT * L, "zot")
        g2 = load(O_G2, GW, "g2")
        ve2 = load(O_VE2, VB2, "ve2")
        g3a = load(O_G3, 2048, "g3a")
        g3b = load(O_G3 + 2048, 2048, "g3b")
        g3c = load(O_G3 + 4096, 2048, "g3c")
        g2x = load(O_G2X, G2XW, "g2x")

        zin = bcat[:, 0 : NT * P]  # [128, 512] z_in transposed (L on part)
        cenb = bcat[:, O_CEN : O_CEN + C]
        ones128 = bcat[:, O_ONE : O_ONE + 1]
        ones10 = bcat[0:1, O_ONE10 : O_ONE10 + C]
        oh = bcat[:, O_OH : O_OH + NT * C]
        eyeI = bcat[:, O_EYEI : O_EYEI + P]
        eyeS = bcat[:, O_EYES : O_EYES + P]
        eye10 = bcat[0:C, O_EYE10 : O_EYE10 + C]
        ohb = bcat[:, O_OHB : O_OHB + NT * C]

        stats = st.tile([P, NSTAT], f32)
        nc.vector.memset(stats[:], 0.0)

        # force the sqrt_and_others ACT table (has sqrt+square+copy+relu)
        # to load once, before any other ACT op picks a different set.
        dsq = sp.tile([1, 1], f32, tag="dsq")
        nc.scalar.activation(dsq[:], stats[0:1, 0:1], Act.Sqrt)

        # ---- z chain, batched ----
        z2 = st.tile([P, NT * P], bf16)
        ps_b = pp.tile([1, NT * P], f32, tag="psB")
        nh = st.tile([1, NT * P], bf16)
        ps_a = pp.tile([C, NT * P], f32, tag="psA")
        sbA = st.tile([C, NT * P], bf16)
        nc.vector.tensor_mul(z2[:], zin, zin)
        nc.tensor.matmul(ps_b[:], lhsT=ones128, rhs=z2[:])
        # nh = -(|z|^2+1)/2
        nc.scalar.activation(nh[:], ps_b[:], Act.Copy, scale=-0.5, bias=-0.5)
        # psA = cen^T zin + ones10 (x) nh  ->  -2*psA = dist^2
        nc.tensor.matmul(ps_a[:], lhsT=cenb, rhs=zin, start=True, stop=False)
        nc.tensor.matmul(ps_a[:], lhsT=ones10, rhs=nh[:], start=False, stop=True)
        nc.scalar.activation(sbA[:], ps_a[:], Act.Copy)

        # orthogonality gram (tiny)
        ps_g = pp.tile([C, C], f32, tag="psG")
        nc.tensor.matmul(ps_g[:], lhsT=cenb, rhs=cenb)

        # transpose dist^2/-2 back to [128 batch, 10] tiles; one sqrt each
        dd = st.tile([P, NT, C], f32)
        for k in range(NT):
            tk = pp.tile([P, C], bf16, tag=f"tk{k}")
            nc.tensor.transpose(tk[:], sbA[:, k * P : (k + 1) * P], eye10)
            nc.scalar.activation(dd[:, k, :], tk[:], Act.Sqrt, scale=-2.0)

        # ---- gram accumulation: one PSUM accumulator over all blocks ----
        G = pp.tile([P, P], f32, tag="G")
        gram_list = [g0, g1, g2, g3a, g3b, g3c, g2x]
        gram_nblk = [BLK012, BLK012, BLK012, 16, 16, 16, BLK2X]
        n_total = sum(gram_nblk)
        gram_pos = [0]

        def gram_chunk(i):
            t = gram_list[i]
            for cb in range(gram_nblk[i]):
                blk = t[:, cb * 128 : (cb + 1) * 128]
                p = gram_pos[0]
                nc.tensor.matmul(
                    G[:],
                    lhsT=blk,
                    rhs=blk,
                    start=(p == 0),
                    stop=(p == n_total - 1),
                )
                gram_pos[0] = p + 1

        # ---- ve chunks: sub on DVE, square-accum on ACT ----
        def ve_chunk(j, t, w):
            df = sp.tile([P, w], bf16, tag=f"df{j}")
            nc.vector.tensor_sub(df[:], t[:, 0:w], t[:, w : 2 * w])
            sq = sp.tile([P, w], bf16, tag=f"sq{j}")
            nc.scalar.activation(
                sq[:], df[:], Act.Square, accum_out=stats[:, C_VE + j : C_VE + j + 1]
            )

        gram_chunk(0)  # g0
        ve_chunk(0, ve0, VE_W01)
        gram_chunk(1)  # g1
        ve_chunk(1, ve1, VE_W01)

        # outlier: |z_out|^2 per row-tile; host computes
        # relu(1 - sqrt(min(n2,1))).
        n2all = st.tile([P, NT], f32)
        for i in range(NT):
            zo = zot[:, i * P : (i + 1) * P]
            zos = sp.tile([P, P], bf16, tag="zos")
            nc.vector.scalar_tensor_tensor(
                out=zos[:], in0=zo, scalar=1.0, in1=zo,
                op0=Alu.mult, op1=Alu.mult,
                accum_out=n2all[:, i : i + 1],
            )
        nc.vector.tensor_scalar_min(stats[:, C_OL : C_OL + NT], n2all[:], 1.0)

        # triplet tail: pos = sum(dd*oh) per tile, neg = min(dd+BIG*oh)-d_in
        s1 = sp.tile([P, NT, C], f32, tag="s1")
        nc.vector.tensor_mul(s1[:], dd[:], oh)
        pos = sp.tile([P, NT], f32, tag="pos")
        nc.vector.tensor_reduce(pos[:], s1[:], axis=mybir.AxisListType.X, op=Alu.add)
        s2 = sp.tile([P, NT, C], f32, tag="s2")
        nc.vector.scalar_tensor_tensor(
            out=s2[:], in0=dd[:], scalar=-D_IN, in1=ohb,
            op0=Alu.add, op1=Alu.add,
        )
        neg = sp.tile([P, NT], f32, tag="neg")
        nc.vector.tensor_reduce(neg[:], s2[:], axis=mybir.AxisListType.X, op=Alu.min)
        vall = sp.tile([P, NT], f32, tag="vall")
        nc.vector.tensor_sub(vall[:], pos[:], neg[:])
        nc.vector.tensor_scalar_max(stats[:, C_TC : C_TC + NT], vall[:], 0.0)

        # orth residual row sums
        gmi = sp.tile([C, C], f32, tag="gmi")
        nc.vector.tensor_sub(gmi[:], ps_g[:], eye10)
        gsc = sp.tile([C, C], f32, tag="gsc")
        nc.vector.scalar_tensor_tensor(
            out=gsc[:], in0=gmi[:], scalar=1.0, in1=gmi[:],
            op0=Alu.mult, op1=Alu.mult,
            accum_out=stats[0:C, C_OR : C_OR + 1],
        )

        gram_chunk(2)  # g2
        ve_chunk(2, ve2, VE_W2)
        gram_chunk(3)  # g3a
        gram_chunk(4)  # g3b
        gram_chunk(5)  # g3c
        gram_chunk(6)  # g2x (last 384 streamed bytes -> 0.2us PE tail)

        # extract gram diagonal (sum x^2 + sum xh^2) and +64
        # off-diagonal (sum x*xh) as per-partition accumulations
        ex = sp.tile([P, P], f32, tag="ex")
        nc.vector.scalar_tensor_tensor(
            out=ex[:], in0=G[:], scalar=1.0, in1=eyeI,
            op0=Alu.mult, op1=Alu.mult,
            accum_out=stats[:, 0:1],
        )
        ex2 = sp.tile([P, P], f32, tag="ex2")
        nc.vector.scalar_tensor_tensor(
            out=ex2[:], in0=G[:], scalar=1.0, in1=eyeS,
            op0=Alu.mult, op1=Alu.mult,
            accum_out=stats[:, 1:2],
        )

        nc.sync.dma_start(out_d[:], stats[:])

    nc.compile()
    return nc


def _get_nc():
    if "nc" not in _CACHE:
        _CACHE["nc"] = _build()
    return _CACHE["nc"]


def _make_in_maps(inputs):
    f8 = ml_dtypes.float8_e4m3fn
    bf = ml_dtypes.bfloat16
    x = np.asarray(inputs["x"], dtype=np.float32)
    xh = np.asarray(inputs["x_hat"], dtype=np.float32)
    zi = np.ascontiguousarray(inputs["z_in"], dtype=np.float32)
    zo = np.ascontiguousarray(inputs["z_out"], dtype=np.float32)
    tgt = np.asarray(inputs["target"]).astype(np.int64)
    cen = np.ascontiguousarray(inputs["center_arr"], dtype=np.float32)

    x8 = x.astype(f8)
    xh8 = xh.astype(f8)

    onehot = np.zeros((B, C), np.float32)
    onehot[np.arange(B), tgt] = 1.0

    norms = np.linalg.norm(cen, axis=1, keepdims=True).astype(np.float32)
    cen_t = np.ascontiguousarray((cen / norms).T.astype(np.float32))

    eyeI = np.eye(P, dtype=np.float32)
    eyeS = np.eye(P, k=64, dtype=np.float32)

    in_maps = []
    for k in range(N_CORES):
        s = slice(k * BS, (k + 1) * BS)
        # [NT, P, D] row-tiled views
        xt = x8[s].reshape(NT, P, D)
        xht = xh8[s].reshape(NT, P, D)

        def gblocks(r, c0, c1):
            # [P, nb, 64]+[P, nb, 64] -> [P, nb*128]
            nb = (c1 - c0) // 64
            a = xt[r, :, c0:c1].reshape(P, nb, 64)
            b = xht[r, :, c0:c1].reshape(P, nb, 64)
            return np.concatenate([a, b], axis=-1).reshape(P, nb * 128)

        def vepack(r, c0, c1):
            return np.concatenate([xt[r, :, c0:c1], xht[r, :, c0:c1]], axis=-1)

        zof = zo[s].reshape(NT, P, L).transpose(1, 0, 2).reshape(P, NT * L)

        xx = np.empty((P, XW), f8)
        xx[:, O_G0 : O_G0 + GW] = gblocks(0, 0, PE_W)
        xx[:, O_VE0 : O_VE0 + VB01] = vepack(0, PE_W, D)
        xx[:, O_G1 : O_G1 + GW] = gblocks(1, 0, PE_W)
        xx[:, O_VE1 : O_VE1 + VB01] = vepack(1, PE_W, D)
        xx[:, O_ZO : O_ZO + NT * L] = zof.astype(f8)
        xx[:, O_G2 : O_G2 + GW] = gblocks(2, 0, PE_W)
        xx[:, O_VE2 : O_VE2 + VB2] = vepack(2, PE_W, PE_W + VE_W2)
        xx[:, O_G3 : O_G3 + G3W] = gblocks(3, 0, D)
        xx[:, O_G2X : O_G2X + G2XW] = gblocks(2, PE_W + VE_W2, D)

        zin_t = zi[s].T  # [L, 512]
        oh3 = onehot[s].reshape(NT, P, C).transpose(1, 0, 2).reshape(P, NT * C)

        bcat = np.ones((P, BW), np.float32)
        bcat[:, O_Z : O_Z + NT * P] = zin_t
        bcat[:, O_CEN : O_CEN + C] = cen_t
        # ones column + ones10 rows stay 1
        bcat[:, O_OH : O_OH + NT * C] = oh3
        bcat[:, O_EYEI : O_EYEI + P] = eyeI
        bcat[:, O_EYES : O_EYES + P] = eyeS
        bcat[:, O_EYE10 : O_EYE10 + C] = 0.0
        bcat[0:C, O_EYE10 : O_EYE10 + C] = np.eye(C, dtype=np.float32)
        bcat[:, O_OHB : O_OHB + NT * C] = oh3 * BIG

        in_maps.append(
            {
                "xx": np.ascontiguousarray(xx),
                "bcat": np.ascontiguousarray(bcat.astype(bf)),
            }
        )
    return in_maps


def _combine(results):
    outs = np.stack([np.asarray(r["out"], dtype=np.float64) for r in results])
    mse_sum = (
        outs[:, :, 0].sum()
        - 2.0 * outs[:, :, 1].sum()
        + outs[:, :, C_VE : C_VE + NVE].sum()
    )
    mse = mse_sum / (B * D)
    tcl = outs[:, :, C_TC : C_TC + NT].sum() / B
    n2c = outs[:, :, C_OL : C_OL + NT]
    ol = np.maximum(1.0 - np.sqrt(n2c), 0.0).sum() / B
    orth = np.sqrt(outs[0, 0:C, C_OR].sum())
    return np.array(np.float32(mse + tcl + ol + orth))


def _run(inputs, trace=False):
    from concourse.bass_utils import run_bass_kernel_spmd

    nc = _get_nc()
    in_maps = _make_in_maps(inputs)
    res = run_bass_kernel_spmd(nc, in_maps, core_ids=list(range(N_CORES)), trace=trace)
    return _combine(res.results), res.exec_time_ns


def kernel(**inputs):
    out, _ = _run(inputs, trace=False)
    return out


def run_traced(inputs):
    """For test.py: returns (output, hw exec_time_ns or None)."""
    return _run(inputs, trace=True)
